# revision 1
# baseline (speedup 1.0000x reference)
"""Trainium2 Bass kernel for nn_ModelNew_3556232922178 (dense_cnn).

Reference computation (B=16, Cin=32, D=H=W=32, Cout=64, k=3):
    y = ConvTranspose3d(x, W, stride=1, pad=0)      # full correlation, out 34^3
    y = (y + bias) * SCALE
    y = (y - running_mean) * rsqrt(running_var+EPS)  # inference BN
    out = y.mean over spatial                        # (B, Cout)

Because the global average pool sums over the *entire* full-correlation
output, every (input voxel, kernel tap) product contributes exactly once:
    sum_spatial(conv)[b,o] = sum_i (sum_spatial x)[b,i] * (sum_taps W)[o,i]
so the whole network collapses to a per-(b,i) spatial reduction of x, a
(B,Cin)x(Cin,Cout) matmul, and a per-channel affine:
    out[b,o] = M[b,o] * alpha[o] + beta[o]
    alpha[o] = SCALE/34^3 * rsqrt(rv[o]+EPS)
    beta[o]  = (bias[o]*SCALE - rm[o]) * rsqrt(rv[o]+EPS)

Sharding: data-parallel over batch, 2 batches per core, 8 cores. Each core
reduces its own x shard (8.4 MB — the dominant, DMA-bound cost), computes
its two output rows completely, no collectives. Host concatenates.

Device layout per core:
  x shard viewed as (2, 128, 8192): partition p = i*4 + q over (channel i,
  spatial quarter q) — a pure host reshape, so every chunk DMA is a
  uniform-partition-stride 2-D AP (compact descriptors; a multi-level
  partition AP makes the trigger engine emit per-row descriptors at
  ~10 us per chunk). Chunk loads alternate between the SP and ACT HWDGE
  queues (~400 GB/s aggregate). Chunked free-axis reduce -> R[(i,q), b]
  (128, 2). Host supplies the tap-reduced W^T replicated over q in
  matching (i,q) order (static-weight preprocessing) and the folded BN
  affine constants alpha/beta. One K=128 PE matmul folds the quarter-sum
  and channel contraction: psum[o, b] = sum_{i,q} Wrep * R. The affine is
  applied with per-partition scalars on (64, 2), DMA'd out; host
  transposes. Chunk reduces are split across the vector engine and the
  scalar engine (activation Copy + accum_out), with per-engine partial
  sums combined by two accumulating matmuls. Measured ~45 us HW span per
  core (DMA-bound; ~23 us is fixed preamble/queue-latency/store-latency/
  teardown overhead present even in raw Bass).
"""

import numpy as np

import concourse.bass as bass
from concourse import mybir
from concourse.tile import TileContext
from concourse.vector_clock import ScopedClock
from concourse.bass_utils import run_bass_kernel_spmd

EPS = 1e-5
SCALE = 2.0
B, CIN, S = 16, 32, 32 * 32 * 32
COUT, KT = 64, 27
NCORES = 8
BPC = B // NCORES          # batches per core
Q = 4                      # spatial quarters -> 128 partitions
F = S // Q                 # 8192 elements per partition per batch
NSPATIAL = 34 * 34 * 34    # conv output positions (pool divisor)
# free-axis chunk sizes per batch: small tails so the last reduce (which
# sits on the critical path after the final chunk lands) is short
CHUNKS_B0 = [2048, 2048, 2048, 2048]
CHUNKS_B1 = [2048, 2048, 2048, 1024, 1024]
F32 = mybir.dt.float32

TRACE = False              # set by test harness to collect an NTFF profile
LAST_RESULT = None         # BassKernelResults of the most recent run


class SplitDrainTileContext(TileContext):
    """TileContext whose exit drain splits sem waits across multiple drains.

    The walrus build here rejects any instruction carrying more than one
    sync wait ("Too many sync wait commands"). Tile's stock exit path puts
    every outstanding proc's wait on a single drain, so any kernel touching
    2+ logical processors fails codegen. Sequential single-wait drains on
    the same engine are semantically identical.
    """

    def _drain_and_barrier(self, tick_clock, wait_clock):
        drain_inst = self.nc.sync.drain()
        wait_clock.add_sem_waits(
            drain_inst.ins, ScopedClock({None: tick_clock.global_clock})
        )
        si = drain_inst.ins.sync_info
        waits = list(si.on_wait) if si is not None and si.on_wait else []
        updates = list(si.on_update) if si is not None and si.on_update else []
        # Poll order matters: each split drain polls its sem sequentially
        # (~0.2 us/poll). Sort so the y store's queue sem (the latest
        # completion: a DMAHW second tick, highest queue name) is polled
        # last — everything else has long completed by then, so no polls
        # remain after the store lands.
        waits.sort(key=lambda w: (w.wait_value, w.ant_name or ""))
        last_drain = drain_inst
        if len(waits) > 1:
            drain_inst.ins.sync_info = mybir.SyncInfo(on_wait=waits[:1], on_update=[])
            for i, w in enumerate(waits[1:]):
                extra = self.nc.sync.drain()
                is_last = i == len(waits) - 2
                extra.ins.sync_info = mybir.SyncInfo(
                    on_wait=[w], on_update=updates if is_last else []
                )
                last_drain = extra

        # Stock Tile brackets the sem reset with two all-engine barriers
        # (leader/follower drains, ~4 us each). The split drains above
        # already wait on every proc's final tick, so a single sem gate
        # (SP drain -> gpsimd clear) gives the same ordering for free.
        # Re-executability is verified by the test harness.
        gate = self.nc.alloc_semaphore("tile_exit_gate")
        last_drain.then_inc(gate, 1)
        self.nc.gpsimd.wait_ge(gate, 1)
        assert self.sems is not None
        popped = self.nc._tile_sem_poison_stack.pop()
        assert popped is self._sem_poison
        self.nc.clear_and_free_semaphores(
            list(self.sems.allocated().values()) + [gate]
        )


def _build_program():
    nc = bass.Bass()
    x = nc.dram_tensor("x", (BPC, 128, F), F32, kind="ExternalInput")
    # Host-prepared tap-reduced W^T replicated over the 4 quarter groups
    # (static-weight preprocessing, same as BN/conv folding):
    # w[(i*4+q), o] = sum_t weight[o, i, t]
    w = nc.dram_tensor("w", (128, COUT), F32, kind="ExternalInput")
    # Host-folded BN affine constants (inference BN folding):
    # ab[:, 0] = SCALE/34^3 * rsqrt(rv+EPS), ab[:, 1] = (bias*SCALE-rm)*rsqrt(rv+EPS)
    ab = nc.dram_tensor("ab", (COUT, 2), F32, kind="ExternalInput")
    y = nc.dram_tensor("y", (COUT, BPC), F32, kind="ExternalOutput")

    with SplitDrainTileContext(nc) as tc:
        with (
            tc.tile_pool(name="const", bufs=1) as const,
            # one slot per chunk: no slot reuse, so chunk DMAs carry no
            # WAR/WAW waits (each instruction may carry at most ONE wait)
            tc.tile_pool(name="xbuf", bufs=len(CHUNKS_B0) + len(CHUNKS_B1)) as xbuf,
            tc.tile_pool(name="ps", bufs=1, space="PSUM") as ps,
        ):
            # Tap-reduced replicated W^T (128, 64) — tiny, via SWDGE.
            wsum = const.tile([128, COUT], F32)
            nc.gpsimd.dma_start(out=wsum, in_=w[:, :])

            # x spatial reduction, chunked for DMA/compute overlap. Triggers
            # split between the SP and ACT HWDGE queues (each trigger costs
            # ~0.6 us of engine time; two queues also engage more DMA
            # engines, ~400 GB/s aggregate vs ~310 on one).
            chunks = []          # (batch, start, size, column)
            col = 0
            for b, sizes in enumerate((CHUNKS_B0, CHUNKS_B1)):
                start = 0
                for sz in sizes:
                    chunks.append((b, start, sz, col))
                    start += sz
                    col += 1
                assert start == F
            ncols = col
            b0_cols = len(CHUNKS_B0)
            stats = const.tile([128, ncols], F32)
            xts = []
            # byte-balanced queue assignment: SP 4x2048, ACT 3x2048 + 2x1024
            engines = [nc.sync, nc.scalar, nc.sync, nc.scalar, nc.sync,
                       nc.scalar, nc.sync, nc.scalar, nc.scalar]
            for k, (b, start, sz, _) in enumerate(chunks):
                xt = xbuf.tile([128, max(max(CHUNKS_B0), max(CHUNKS_B1))], F32)
                engines[k].dma_start(out=xt[:, :sz], in_=x[b, :, start : start + sz])
                xts.append(xt)
            ab_t = const.tile([COUT, 2], F32)
            nc.gpsimd.dma_start(out=ab_t, in_=ab[:, :])
            wsum_s = const.tile([128, COUT], F32)
            ab_s = const.tile([COUT, 2], F32)
            # DVE alone needs ~18.5 us for all reduces (co-critical with the
            # DMA window), so chunks k1,k3,k5 are reduced on ACT instead via
            # activation(Copy, accum_out=sum). Separate stats tiles per
            # engine keep every consumer at one sem wait; the per-engine
            # partial sums are combined by two accumulating matmuls.
            ACT_CHUNKS = {1, 3, 5}
            dve_cols = [c for _, _, _, c in chunks if c not in ACT_CHUNKS]
            act_cols = [c for _, _, _, c in chunks if c in ACT_CHUNKS]
            stats_d = const.tile([128, len(dve_cols)], F32)
            stats_a = const.tile([128, len(act_cols)], F32)
            act_scratches = [
                const.tile(
                    [128, max(max(CHUNKS_B0), max(CHUNKS_B1))],
                    F32,
                    name=f"act_scratch{i}",
                )
                for i in range(len(act_cols))
            ]
            dve_col_of = {c: i for i, c in enumerate(dve_cols)}
            act_col_of = {c: i for i, c in enumerate(act_cols)}
            ndve = 0
            for j, ((b, start, sz, c), xt) in enumerate(zip(chunks, xts)):
                if c in ACT_CHUNKS:
                    i = act_col_of[c]
                    nc.scalar.activation(
                        out=act_scratches[i][:, :sz],
                        in_=xt[:, :sz],
                        func=mybir.ActivationFunctionType.Copy,
                        accum_out=stats_a[:, i : i + 1],
                    )
                else:
                    i = dve_col_of[c]
                    nc.vector.reduce_sum(
                        out=stats_d[:, i : i + 1],
                        in_=xt[:, :sz],
                        axis=mybir.AxisListType.X,
                    )
                    ndve += 1
                    if ndve == 2:
                        # DVE-side copies of the small SWDGE inputs (waits
                        # long satisfied) so matmul/affine operands are
                        # DVE-produced and carry a single sem wait.
                        nc.vector.tensor_copy(wsum_s, wsum)
                        nc.vector.tensor_copy(ab_s, ab_t)
            # per-batch partial sums per engine (batch boundary = b0_cols)
            d_split = sum(1 for c in dve_cols if c < b0_cols)
            a_split = sum(1 for c in act_cols if c < b0_cols)
            red_d = const.tile([128, BPC], F32)
            red_a = const.tile([128, BPC], F32)
            nc.vector.reduce_sum(
                out=red_d[:, 0:1], in_=stats_d[:, 0:d_split], axis=mybir.AxisListType.X
            )
            nc.vector.reduce_sum(
                out=red_d[:, 1:2], in_=stats_d[:, d_split:], axis=mybir.AxisListType.X
            )
            nc.vector.reduce_sum(
                out=red_a[:, 0:1], in_=stats_a[:, 0:a_split], axis=mybir.AxisListType.X
            )
            nc.vector.reduce_sum(
                out=red_a[:, 1:2], in_=stats_a[:, a_split:], axis=mybir.AxisListType.X
            )

            # psum[o, b] = sum_{(q,i)} wsum[(q,i), o] * (red_d + red_a) via
            # two accumulating K=128 matmuls (folds quarter-sum + channels).
            pm = ps.tile([COUT, BPC], F32)
            nc.tensor.matmul(pm, wsum_s, red_d, start=True, stop=False)
            nc.tensor.matmul(pm, wsum_s, red_a, start=False, stop=True)

            out_t = const.tile([COUT, BPC], F32)
            nc.vector.tensor_scalar(                            # waits PE only
                out=out_t,
                in0=pm,
                scalar1=ab_s[:, 0:1],
                scalar2=ab_s[:, 1:2],
                op0=mybir.AluOpType.mult,
                op1=mybir.AluOpType.add,
            )
            # ACT HWDGE store (lower completion latency than SWDGE). As the
            # 10th HWDGE DMA it wraps the 8-proc round robin and picks up a
            # DMAHW wait that its DVE wait transitively implies; stripped
            # post-build in _elide_y_store_wrap_wait.
            nc.scalar.dma_start(out=y[:, :], in_=out_t)

    _elide_y_store_wrap_wait(nc)
    return nc


def _elide_y_store_wrap_wait(nc):
    """Drop the DMAHW proc-wrap wait from the y store.

    The store's only data dependency is out_t (DVE). Its DMAHW wait exists
    because Tile's 8 HWDGE proc slots wrapped; the proc's earlier DMA is an
    x chunk whose reduce -> red -> matmul -> affine chain precedes out_t,
    so the wait is transitively implied and safe to elide (the codegen
    rejects instructions with more than one sem wait).
    """
    stripped = 0
    for f in nc.m.functions:
        for bb in f.blocks:
            for inst in bb.instructions:
                si = inst.sync_info
                if si is None or not si.on_wait or len(si.on_wait) < 2:
                    continue
                names = [w.ant_name or "" for w in si.on_wait]
                keep = [
                    w for w in si.on_wait if not (w.ant_name or "").startswith("DMAHW")
                ]
                assert len(keep) == 1 and keep[0].ant_name.startswith("DVE"), names
                inst.sync_info = mybir.SyncInfo(
                    on_wait=keep, on_update=list(si.on_update or [])
                )
                stripped += 1
    assert stripped <= 1, f"expected at most the y store, stripped {stripped}"


def prep_inputs(x, weight, bias, running_mean, running_var):
    """Host-side sharding prep: per-core in_maps for run_bass_kernel_spmd."""
    x = np.ascontiguousarray(np.asarray(x, dtype=np.float32))
    weight = np.ascontiguousarray(np.asarray(weight, dtype=np.float32))
    bias = np.ascontiguousarray(np.asarray(bias, dtype=np.float32))
    rm = np.ascontiguousarray(np.asarray(running_mean, dtype=np.float32))
    rv = np.ascontiguousarray(np.asarray(running_var, dtype=np.float32))

    xv = x.reshape(B, 128, F)          # (b, i*4+q, f) — contiguous view
    # Static weight preprocessing (BN/conv-fold style): tap-reduce W and
    # replicate W^T across the 4 quarter groups, i-outer to match x (32 KB)
    wv = np.ascontiguousarray(
        np.repeat(
            weight.reshape(COUT, CIN, KT).sum(axis=2).T.astype(np.float32), Q, axis=0
        )
    )
    rstd = (1.0 / np.sqrt(rv + np.float32(EPS))).astype(np.float32)
    alpha = (np.float32(SCALE / NSPATIAL) * rstd).astype(np.float32)
    beta = ((bias * np.float32(SCALE) - rm) * rstd).astype(np.float32)
    ab = np.ascontiguousarray(np.stack([alpha, beta], axis=1))
    return [
        {"x": xv[k * BPC : (k + 1) * BPC], "w": wv, "ab": ab}
        for k in range(NCORES)
    ]


def kernel(x, weight, bias, running_mean, running_var):
    global LAST_RESULT
    in_maps = prep_inputs(x, weight, bias, running_mean, running_var)
    nc = _build_program()
    res = run_bass_kernel_spmd(
        nc, in_maps, core_ids=list(range(NCORES)), trace=TRACE
    )
    LAST_RESULT = res

    out = np.empty((B, COUT), dtype=np.float32)
    for k in range(NCORES):
        out[k * BPC : (k + 1) * BPC] = res.results[k]["y"].T
    return out

